# revision 19
# baseline (speedup 1.0000x reference)
"""MiniLCX retrieval-KNN kernel for Trainium2 (8 NeuronCores).

Contract: kernel(**inputs) takes the FULL unsharded inputs (as produced by
setup_inputs()) and returns the full outputs
(context, margin, top_slot, new_keys, new_values).

Strategy:
  - Shard the hidden batch N=8192 across 8 cores (1024 rows each); every core
    sees the full slot set S=32768.
  - Host prep: transpose layouts (hiddenT shard, w_q.T) and fold the key
    normalization + validity mask into a pre-scaled keysT operand
    (columns of masked slots are exactly 0, and 0 never survives top-8 since
    the 8th best of ~16k valid cosine scores is > 0 w.o.p.).
  - Device per core: qT = normalize(w_q @ hiddenT + b_q) folded into the
    stationary matmul operand; scores streamed block-by-block (4 blocks of
    8192 slots, keys block resident in SBUF); per (row-tile, block) top-8 via
    the DVE max8/max_index instructions; block top-8s merged on-device; softmax
    + indirect-DMA gather of values rows + weighted sum -> context.
    Partial sums for the EMA write path (write_key / write_val / gate) are
    computed on-device; the final scalar combine + one-row EMA write into
    keys/values happens on host (O(S*d) copy, not compute).
"""

import os
import sys

for _p in ("/opt/trn_rl_repo", "/opt/pypackages"):
    if _p not in sys.path:
        sys.path.insert(0, _p)

import ml_dtypes
import numpy as np

import concourse.bass as bass
import concourse.mybir as mybir
from concourse import bacc
from concourse import tile as tile_mod
from concourse.bass import IndirectOffsetOnAxis
from concourse.bass_utils import run_bass_kernel_spmd

P = 128
F32 = mybir.dt.float32
F32R = mybir.dt.float32r
BF16 = mybir.dt.bfloat16
I32 = mybir.dt.int32
U32 = mybir.dt.uint32
ALU = mybir.AluOpType
ACT = mybir.ActivationFunctionType
AX = mybir.AxisListType

# problem shape (hardcoded per the harness contract)
N, D, KD, S = 8192, 1024, 256, 32768
TOPK = 8
NCORES = 8


def build_nc(rows, s_total, nblk, stage=4):
    """Build the per-core Bass program.

    rows: rows handled by this core (N // NCORES)
    s_total: total slots (S)
    nblk: number of key blocks (block = s_total // nblk slots, must be
          a multiple of 1024 and <= 16384 for max8)
    """
    assert rows % P == 0
    nrt = rows // P                  # row tiles
    blk = s_total // nblk            # slots per block
    nhalf = 2                        # half-blocks per block (DMA granularity)
    half = blk // nhalf
    assert half % 512 == 0
    nch = half // 512                # 512-wide matmul chunks per half-block
    dt_tiles = D // P                # 8
    kt_tiles = KD // P               # 2
    rchunk = min(512, rows)          # row chunking for phase-1 matmuls
    nrch = rows // rchunk

    nc = bacc.Bacc("TRN2", target_bir_lowering=False, debug=False)

    # ---------------- DRAM I/O ----------------
    hT_d = nc.dram_tensor("hT", (P, dt_tiles, rows), F32, kind="ExternalInput")
    wqT_d = nc.dram_tensor("wqT", (P, dt_tiles, KD), F32, kind="ExternalInput")
    bqT_d = nc.dram_tensor("bqT", (P, kt_tiles), F32, kind="ExternalInput")
    wgT_d = nc.dram_tensor("wgT", (P, dt_tiles), F32, kind="ExternalInput")
    bg_d = nc.dram_tensor("bg", (1, 1), F32, kind="ExternalInput")
    keysTnh_d = nc.dram_tensor(
        "keysTn_hi", (kt_tiles, P, s_total), BF16, kind="ExternalInput"
    )
    keysTnl_d = nc.dram_tensor(
        "keysTn_lo", (kt_tiles, P, s_total), BF16, kind="ExternalInput"
    )
    values_d = nc.dram_tensor("values", (s_total, D), F32, kind="ExternalInput")
    iota32_d = nc.dram_tensor("iota32", (P, nblk * TOPK), F32, kind="ExternalInput")

    ctx_d = nc.dram_tensor("context", (nrt, P, D), F32, kind="ExternalOutput")
    ts_d = nc.dram_tensor("topk_scores", (nrt, P, TOPK), F32, kind="ExternalOutput")
    ti_d = nc.dram_tensor("topk_idx", (nrt, P, TOPK), I32, kind="ExternalOutput")
    wkey_d = nc.dram_tensor("wkey_sum", (P, kt_tiles), F32, kind="ExternalOutput")
    wval_d = nc.dram_tensor("wval_sum", (P, dt_tiles), F32, kind="ExternalOutput")
    gate_d = nc.dram_tensor("gate_sum", (1, 1), F32, kind="ExternalOutput")

    with tile_mod.TileContext(nc) as tc:
        # persistent pools
        with (
            tc.tile_pool(name="persist", bufs=1) as persist,
            tc.tile_pool(name="cand", bufs=1) as candp,
        ):
            qhi = persist.tile([P, kt_tiles, rows], BF16, tag="qhi")
            qlo = persist.tile([P, kt_tiles, rows], BF16, tag="qlo")
            iota32 = persist.tile([P, nblk * TOPK], F32, tag="iota32")
            nc.sync.dma_start(iota32[:], iota32_d[:])

            # ---------------- phase 1: q / gate / write partials ------------
            with (
                tc.tile_pool(name="ph1", bufs=1) as ph1,
                tc.tile_pool(name="ph1ps", bufs=4, space="PSUM") as ph1ps,
                tc.tile_pool(name="ph1ps1", bufs=2, space="PSUM") as ph1ps1,
            ):
                hT = ph1.tile([P, dt_tiles, rows], F32, tag="hT")
                wqT = ph1.tile([P, dt_tiles, KD], F32, tag="wqT")
                bqT = ph1.tile([P, kt_tiles], F32, tag="bqT")
                wgT = ph1.tile([P, dt_tiles], F32, tag="wgT")
                bg = ph1.tile([1, 1], F32, tag="bg")
                ones = ph1.tile([P, 1], F32, tag="ones")
                qTp = ph1.tile([P, kt_tiles, rows], F32, tag="qTp")
                sq = ph1.tile([P, kt_tiles, rows], F32, tag="sq")
                qinv = ph1.tile([1, rows], F32, tag="qinv")
                qnrm = ph1.tile([1, rows], F32, tag="qnrm")
                qinvb = ph1.tile([P, rows], F32, tag="qinvb")
                wkey = ph1.tile([P, kt_tiles], F32, tag="wkey")
                wval = ph1.tile([P, dt_tiles], F32, tag="wval")
                sgt = ph1.tile([1, rows], F32, tag="sgt")
                gsum = ph1.tile([1, 1], F32, tag="gsum")

                nc.sync.dma_start(hT[:], hT_d[:])
                nc.sync.dma_start(wqT[:], wqT_d[:])
                nc.sync.dma_start(bqT[:], bqT_d[:])
                nc.sync.dma_start(wgT[:], wgT_d[:])
                nc.sync.dma_start(bg[:], bg_d[:])
                nc.vector.memset(ones[:], 1.0)

                # qT_pre = w_q @ hiddenT + b_q   (layout [kd, rows])
                for kt in range(kt_tiles):
                    for ch in range(nrch):
                        ps = ph1ps.tile([P, rchunk], F32, tag="qps")
                        for dt in range(dt_tiles):
                            nc.tensor.matmul(
                                ps[:],
                                lhsT=wqT[:, dt, kt * P : (kt + 1) * P],
                                rhs=hT[:, dt, ch * rchunk : (ch + 1) * rchunk],
                                start=(dt == 0),
                                stop=(dt == dt_tiles - 1),
                            )
                        # copy + bias (per-partition bias AP)
                        nc.scalar.activation(
                            out=qTp[:, kt, ch * rchunk : (ch + 1) * rchunk],
                            in_=ps[:],
                            func=ACT.Identity,
                            bias=bqT[:, kt : kt + 1],
                        )

                # write_key partial: sum of q_pre over rows
                nc.vector.tensor_reduce(
                    out=wkey[:], in_=qTp[:], axis=AX.X, op=ALU.add
                )
                nc.sync.dma_start(wkey_d[:], wkey[:])

                # write_val partial: sum of hidden over rows
                nc.vector.tensor_reduce(
                    out=wval[:], in_=hT[:], axis=AX.X, op=ALU.add
                )
                nc.sync.dma_start(wval_d[:], wval[:])

                # q row norms: sumsq over kd (cross-partition via ones-matmul)
                nc.scalar.activation(out=sq[:], in_=qTp[:], func=ACT.Square)
                for ch in range(nrch):
                    ps1 = ph1ps1.tile([1, rchunk], F32, tag="nps")
                    for kt in range(kt_tiles):
                        nc.tensor.matmul(
                            ps1[:],
                            lhsT=ones[:],
                            rhs=sq[:, kt, ch * rchunk : (ch + 1) * rchunk],
                            start=(kt == 0),
                            stop=(kt == kt_tiles - 1),
                        )
                    nc.scalar.activation(
                        out=qnrm[:, ch * rchunk : (ch + 1) * rchunk],
                        in_=ps1[:],
                        func=ACT.Sqrt,
                    )
                nc.vector.reciprocal(qinv[:], qnrm[:])

                # qTn = qT_pre * qinv (broadcast over partitions via gpsimd),
                # then split into bf16 hi + lo for the 3-pass scores matmul.
                nc.gpsimd.partition_broadcast(qinvb[:], qinv[:])
                qTn = ph1.tile([P, kt_tiles, rows], F32, tag="qTn")
                qhif = ph1.tile([P, kt_tiles, rows], F32, tag="qhif")
                for kt in range(kt_tiles):
                    nc.vector.tensor_tensor(
                        out=qTn[:, kt, :],
                        in0=qTp[:, kt, :],
                        in1=qinvb[:],
                        op=ALU.mult,
                    )
                nc.vector.tensor_copy(qhi[:], qTn[:])
                nc.vector.tensor_copy(qhif[:], qhi[:])
                nc.vector.tensor_tensor(
                    out=qTn[:], in0=qTn[:], in1=qhif[:], op=ALU.subtract
                )
                nc.vector.tensor_copy(qlo[:], qTn[:])

                # gate partial: sum(sigmoid(hidden @ w_g.T + b_g))
                for ch in range(nrch):
                    ps1 = ph1ps1.tile([1, rchunk], F32, tag="gps")
                    for dt in range(dt_tiles):
                        nc.tensor.matmul(
                            ps1[:],
                            lhsT=wgT[:, dt : dt + 1],
                            rhs=hT[:, dt, ch * rchunk : (ch + 1) * rchunk],
                            start=(dt == 0),
                            stop=(dt == dt_tiles - 1),
                        )
                    nc.scalar.activation(
                        out=sgt[:, ch * rchunk : (ch + 1) * rchunk],
                        in_=ps1[:],
                        func=ACT.Sigmoid,
                        bias=bg[0:1, 0:1],
                    )
                nc.vector.tensor_reduce(
                    out=gsum[:], in_=sgt[:], axis=AX.X, op=ALU.add
                )
                nc.sync.dma_start(gate_d[:], gsum[:])

            # per-row-tile candidate buffers (persist across blocks)
            cands = []
            for rt in range(nrt):
                cv = candp.tile([P, nblk * TOPK], F32, tag=f"cv{rt}")
                ci = candp.tile([P, nblk * TOPK], F32, tag=f"ci{rt}")
                cands.append((cv, ci))

            # ---------------- phase 2: scores + top-k + context -------------
            with (
                tc.tile_pool(name="keyp", bufs=3) as keyp,
                tc.tile_pool(name="scorep", bufs=2) as scorep,
                tc.tile_pool(name="mainps", bufs=8, space="PSUM") as mainps,
                tc.tile_pool(name="smallp", bufs=2) as smallp,
                tc.tile_pool(name="vgp", bufs=2) as vgp,
                tc.tile_pool(name="ctxp", bufs=2) as ctxp,
            ):

                def merge_and_context(rt):
                    cv, ci = cands[rt]
                    ts8 = smallp.tile([P, TOPK], F32, tag="ts8")
                    pos = smallp.tile([P, TOPK], U32, tag="pos")
                    posf = smallp.tile([P, TOPK], F32, tag="posf")
                    slotf = smallp.tile([P, TOPK], F32, tag="slotf")
                    sloti = smallp.tile([P, TOPK], I32, tag="sloti")
                    eq = smallp.tile([P, nblk * TOPK], F32, tag="eq")
                    scr = smallp.tile([P, nblk * TOPK], F32, tag="scr")
                    negm = smallp.tile([P, 1], F32, tag="negm")
                    ew = smallp.tile([P, TOPK], F32, tag="ew")
                    esum = smallp.tile([P, 1], F32, tag="esum")
                    rin = smallp.tile([P, 1], F32, tag="rin")
                    w8 = smallp.tile([P, TOPK], F32, tag="w8")

                    nc.vector.max(out=ts8[:], in_=cv[:])
                    nc.vector.max_index(out=pos[:], in_max=ts8[:], in_values=cv[:])
                    nc.vector.tensor_copy(posf[:], pos[:])
                    if stage == 31:
                        nc.vector.tensor_copy(sloti[:], posf[:])
                        nc.vector.tensor_copy(w8[:], ts8[:])
                        nc.sync.dma_start(ts_d[rt], ts8[:])
                        nc.sync.dma_start(ti_d[rt], sloti[:])
                        return
                    # slot index extraction: slotf[:,k] = ci[pos[k]] via one-hot dot
                    nv = nblk * TOPK
                    for k in range(TOPK):
                        if stage == 33 or stage >= 4:
                            nc.vector.tensor_tensor(
                                out=eq[:],
                                in0=iota32[:],
                                in1=posf[:, k : k + 1].to_broadcast([P, nv]),
                                op=ALU.is_equal,
                            )
                            nc.vector.tensor_tensor(
                                out=scr[:], in0=eq[:], in1=ci[:], op=ALU.mult
                            )
                            nc.vector.tensor_reduce(
                                out=slotf[:, k : k + 1], in_=scr[:],
                                axis=AX.X, op=ALU.add,
                            )
                        else:
                            nc.vector.tensor_scalar(
                                eq[:],
                                iota32[:],
                                posf[:, k : k + 1],
                                None,
                                op0=ALU.is_equal,
                            )
                            nc.vector.tensor_tensor_reduce(
                                out=scr[:],
                                in0=eq[:],
                                in1=ci[:],
                                scale=1.0,
                                scalar=0.0,
                                op0=ALU.mult,
                                op1=ALU.add,
                                accum_out=slotf[:, k : k + 1],
                            )
                    nc.vector.tensor_copy(sloti[:], slotf[:])
                    if stage in (32, 33):
                        nc.sync.dma_start(ts_d[rt], ts8[:])
                        nc.sync.dma_start(ti_d[rt], sloti[:])
                        return

                    # softmax over the 8 scores
                    nc.vector.tensor_scalar(
                        negm[:], ts8[:, 0:1], -1.0, None, op0=ALU.mult
                    )
                    nc.scalar.activation(
                        out=ew[:],
                        in_=ts8[:],
                        func=ACT.Exp,
                        bias=negm[:, 0:1],
                        accum_out=esum[:],
                    )
                    nc.vector.reciprocal(rin[:], esum[:])
                    nc.vector.tensor_scalar(
                        w8[:], ew[:], rin[:, 0:1], None, op0=ALU.mult
                    )

                    nc.sync.dma_start(ts_d[rt], ts8[:])
                    nc.sync.dma_start(ti_d[rt], sloti[:])
                    if stage < 4:
                        return
                    # context = sum_k w8[k] * values[slot[k]]
                    ctx = ctxp.tile([P, D], F32, tag="ctx")
                    for k in range(TOPK):
                        vg = vgp.tile([P, D], F32, tag="vg")
                        nc.gpsimd.indirect_dma_start(
                            out=vg[:],
                            out_offset=None,
                            in_=values_d[:],
                            in_offset=IndirectOffsetOnAxis(
                                ap=sloti[:, k : k + 1], axis=0
                            ),
                        )
                        if k == 0:
                            nc.vector.tensor_scalar(
                                ctx[:], vg[:], w8[:, 0:1], None, op0=ALU.mult
                            )
                        else:
                            nc.vector.scalar_tensor_tensor(
                                out=ctx[:],
                                in0=vg[:],
                                scalar=w8[:, k : k + 1],
                                in1=ctx[:],
                                op0=ALU.mult,
                                op1=ALU.add,
                            )
                    nc.sync.dma_start(ctx_d[rt], ctx[:])

                for b in range(nblk if stage >= 2 else 0):
                    kbs = []
                    for h in range(nhalf):
                        kbh = keyp.tile([P, kt_tiles, half], BF16, tag="keyhbh")
                        kbl = keyp.tile([P, kt_tiles, half], BF16, tag="keyhbl")
                        off = b * blk + h * half
                        for kt in range(kt_tiles):
                            nc.sync.dma_start(
                                kbh[:, kt, :], keysTnh_d[kt, :, off : off + half]
                            )
                            nc.sync.dma_start(
                                kbl[:, kt, :], keysTnl_d[kt, :, off : off + half]
                            )
                        kbs.append((kbh, kbl))
                    for rt in range(nrt):
                        sb = scorep.tile([P, blk], F32, tag="scb")
                        for h in range(nhalf):
                            kbh, kbl = kbs[h]
                            pss = [
                                mainps.tile([P, 512], F32, tag="sps", name=f"sps{c}")
                                for c in range(nch)
                            ]
                            terms = [(qhi, kbh), (qhi, kbl), (qlo, kbh)]
                            for ti, (qt, kt_b) in enumerate(terms):
                                for kt in range(kt_tiles):
                                    for c in range(nch):
                                        nc.tensor.matmul(
                                            pss[c][:],
                                            lhsT=qt[:, kt, rt * P : (rt + 1) * P],
                                            rhs=kt_b[:, kt, c * 512 : (c + 1) * 512],
                                            start=(ti == 0 and kt == 0),
                                            stop=(
                                                ti == len(terms) - 1
                                                and kt == kt_tiles - 1
                                            ),
                                        )
                            for c in range(nch):
                                nc.scalar.activation(
                                    out=sb[
                                        :,
                                        h * half + c * 512 : h * half + (c + 1) * 512,
                                    ],
                                    in_=pss[c][:],
                                    func=ACT.Copy,
                                )
                        cv, ci = cands[rt]
                        bidx = smallp.tile([P, TOPK], U32, tag="bidx")
                        bidxf = smallp.tile([P, TOPK], F32, tag="bidxf")
                        nc.vector.max(
                            out=cv[:, b * TOPK : (b + 1) * TOPK], in_=sb[:]
                        )
                        nc.vector.max_index(
                            out=bidx[:],
                            in_max=cv[:, b * TOPK : (b + 1) * TOPK],
                            in_values=sb[:],
                        )
                        nc.vector.tensor_copy(bidxf[:], bidx[:])
                        nc.vector.tensor_scalar(
                            ci[:, b * TOPK : (b + 1) * TOPK],
                            bidxf[:],
                            float(b * blk),
                            None,
                            op0=ALU.add,
                        )
                        if b == nblk - 1 and stage >= 3:
                            merge_and_context(rt)


    nc.compile()
    return nc


_NC_CACHE = {}
LAST_RESULTS = None


def _install_ntff_hook_shim():
    """Register the axon NTFF profile hook when antenv.axon_hooks is absent."""
    import types

    try:
        from antenv.axon_hooks import get_axon_ntff_profile_hook  # noqa: F401
        return
    except ImportError:
        pass
    try:
        sys.path.insert(0, "/root/.axon_site")
        from trn_agent_boot.trn_boot import _ntff_profile_via_ctypes

        hook = _ntff_profile_via_ctypes("/opt/axon/libaxon_pjrt.so")
        mod = types.ModuleType("antenv.axon_hooks")
        mod.get_axon_ntff_profile_hook = lambda: hook
        mod.set_axon_ntff_profile_hook = lambda h: None
        sys.modules["antenv.axon_hooks"] = mod
    except Exception as e:  # pragma: no cover
        print("ntff hook shim failed:", e)


def _get_nc():
    key = (N // NCORES, S, 4)
    if key not in _NC_CACHE:
        _NC_CACHE[key] = build_nc(*key)
    return _NC_CACHE[key]


def kernel(hidden, w_q, b_q, w_g, b_g, keys, values, valid):
    hidden = np.asarray(hidden, dtype=np.float32)
    w_q = np.asarray(w_q, dtype=np.float32)
    b_q = np.asarray(b_q, dtype=np.float32)
    w_g = np.asarray(w_g, dtype=np.float32)
    b_g = np.asarray(b_g, dtype=np.float32)
    keys = np.asarray(keys, dtype=np.float32)
    values = np.asarray(values, dtype=np.float32)
    valid = np.asarray(valid)

    rows = N // NCORES
    dt_tiles, kt_tiles = D // P, KD // P

    # ---- host-side layout prep (sharding + transposes + key norm fold) ----
    eff = valid | (not bool(valid.any()))
    norms = np.linalg.norm(keys, axis=1)
    kinv = (eff.astype(np.float32) / np.maximum(norms, 1e-12)).astype(np.float32)
    # keysTn[kt, p, s] = keys[s, kt*128+p] * kinv[s], split bf16 hi + lo
    keysTn = np.ascontiguousarray(
        (keys * kinv[:, None]).T.reshape(kt_tiles, P, S)
    )
    keysTn_hi = keysTn.astype(ml_dtypes.bfloat16)
    keysTn_lo = (keysTn - keysTn_hi.astype(np.float32)).astype(ml_dtypes.bfloat16)
    wqT = np.ascontiguousarray(w_q.T.reshape(dt_tiles, P, KD).transpose(1, 0, 2))
    bqT = np.ascontiguousarray(b_q.reshape(kt_tiles, P).T)
    wgT = np.ascontiguousarray(w_g[0].reshape(dt_tiles, P).T)
    bg = b_g.reshape(1, 1).astype(np.float32)
    iota32 = np.broadcast_to(
        np.arange(32, dtype=np.float32), (P, 32)
    ).copy()

    in_maps = []
    for c in range(NCORES):
        hsh = hidden[c * rows : (c + 1) * rows]
        # hT[p, dt, n] = hidden[n, dt*128+p]
        hT = np.ascontiguousarray(
            hsh.T.reshape(dt_tiles, P, rows).transpose(1, 0, 2)
        )
        in_maps.append(
            {
                "hT": hT,
                "wqT": wqT,
                "bqT": bqT,
                "wgT": wgT,
                "bg": bg,
                "keysTn_hi": keysTn_hi,
                "keysTn_lo": keysTn_lo,
                "values": values,
                "iota32": iota32,
            }
        )

    nc = _get_nc()
    trace = bool(int(os.environ.get("BASSKNN_TRACE", "0")))
    if trace:
        _install_ntff_hook_shim()
    res = run_bass_kernel_spmd(
        nc, in_maps, core_ids=list(range(NCORES)), trace=trace
    )
    global LAST_RESULTS
    LAST_RESULTS = res
    outs = res.results

    # ---- gather / finalize on host ----
    context = np.concatenate(
        [o["context"].reshape(rows, D) for o in outs], axis=0
    )
    topk_scores = np.concatenate(
        [o["topk_scores"].reshape(rows, TOPK) for o in outs], axis=0
    )
    topk_idx = np.concatenate(
        [o["topk_idx"].reshape(rows, TOPK) for o in outs], axis=0
    )
    top_slot = topk_idx[:, 0].astype(np.int32)

    if np.any(np.isinf(topk_scores)):
        margin = np.float32(0.0)
    else:
        margin = np.float32(
            np.mean(topk_scores[:, 0] - topk_scores[:, 1], dtype=np.float64)
        )

    wkey_sum = np.sum(
        [o["wkey_sum"] for o in outs], axis=0, dtype=np.float64
    )  # [128, 2]
    wval_sum = np.sum(
        [o["wval_sum"] for o in outs], axis=0, dtype=np.float64
    )  # [128, 8]
    gate_sum = np.sum([o["gate_sum"] for o in outs], dtype=np.float64)

    gate = np.float32(gate_sum / N)
    write_val = (wval_sum.T.reshape(D) / N).astype(np.float32)
    wk = (wkey_sum.T.reshape(KD) / N).astype(np.float32)
    write_key = (wk / max(np.linalg.norm(wk), 1e-12)).astype(np.float32)

    slot = int(top_slot[0])
    new_keys = keys.copy()
    new_values = values.copy()
    new_keys[slot] = (1.0 - gate) * keys[slot] + gate * write_key
    new_values[slot] = (1.0 - gate) * values[slot] + gate * write_val

    return context, margin, top_slot, new_keys, new_values


# expose the last run's perf info for test.py
def kernel_with_perf(**inputs):
    out = kernel(**inputs)
    return out
